# revision 1
# baseline (speedup 1.0000x reference)
"""Paged-attention decode kernel (rmsnorm + neox-rope + cache update + GQA attention)
for Trainium2, sharded tensor-parallel over the 8 KV heads across 8 NeuronCores.

Per core (kv head h): 4 query heads, 64 seqs, context 512..1024 tokens.
Pipeline per sequence:
  - K tiles loaded from the paged cache with an f32->bf16 casting SWDGE DMA,
    new-token row patched in SBUF, transposed on the PE (bf16) to KT [d, t]
  - scores = qT.T @ KT accumulated in PSUM, 4 seqs packed per PSUM bank pair
    at partition bases 0/32/64/96 (explicit tile_position)
  - exp on ACT (f32 psum -> bf16 E, f32 row-sum accumulator)
  - E densified via SBUF->SBUF DMA (cross-partition pack), transposed on PE
    to PT [t, (s,g)], PV = PT.T @ V accumulated per seq in PSUM
  - extraction applies 1/rowsum (flash-style late normalization)
Softmax max-subtraction is skipped: q/k are rms-normalized so |scores| <~ 6
and exp stays comfortably in f32/bf16 range.
"""
import os
import numpy as np

S = 64            # sequences
NH = 32           # query heads
KVH = 8           # kv heads
G = NH // KVH     # query heads per kv head (4)
D = 128           # head dim
BS = 16           # cache block size
MAXLEN = 1024
BPS = MAXLEN // BS    # blocks per seq
P = 128
SCALE = 1.0 / float(np.sqrt(D))
EPS = 1e-6

_cache = {}


def _build(nt, pp, row0, Ls, n_cores_unused=None):
    """Build + compile the SPMD program for one core (identical across cores).

    nt[s]:   number of 128-token tiles for seq s (ceil(L/128))
    pp[s]:   partition (token % 128) of the new token within its tile
    row0[s]: first cache row of seq s (block_tables[s,0]*16)
    """
    import concourse.bacc as bacc
    import concourse.mybir as mybir
    import concourse.tile as tile
    from concourse.masks import make_identity

    F32 = mybir.dt.float32
    BF = mybir.dt.bfloat16
    I32 = mybir.dt.int32
    AF = mybir.ActivationFunctionType

    nc = bacc.Bacc("TRN2", target_bir_lowering=False)
    q_h = nc.declare_dram_parameter("q_h", [S * G, D], F32, isOutput=False)
    k_h = nc.declare_dram_parameter("k_h", [S, D], F32, isOutput=False)
    v_h = nc.declare_dram_parameter("v_h", [S, D], F32, isOutput=False)
    kc = nc.declare_dram_parameter("kc", [S * MAXLEN, D], F32, isOutput=False)
    vc = nc.declare_dram_parameter("vc", [S * MAXLEN, D], F32, isOutput=False)
    qw2 = nc.declare_dram_parameter("qw2", [1, D], F32, isOutput=False)
    kw2 = nc.declare_dram_parameter("kw2", [1, D], F32, isOutput=False)
    cosc = nc.declare_dram_parameter("cosc", [MAXLEN, D // 2], F32, isOutput=False)
    sinc = nc.declare_dram_parameter("sinc", [MAXLEN, D // 2], F32, isOutput=False)
    posr = nc.declare_dram_parameter("posr", [S * G, 1], I32, isOutput=False)
    pos1 = nc.declare_dram_parameter("pos1", [S, 1], I32, isOutput=False)
    outp = nc.declare_dram_parameter("out", [S * G, D], F32, isOutput=True)

    H = D // 2

    with tile.TileContext(nc) as tc:
        with tc.tile_pool(name="single", bufs=1) as single, \
             tc.tile_pool(name="sb", bufs=5) as sb, \
             tc.tile_pool(name="ktsp", bufs=5) as ktsp, \
             tc.tile_pool(name="vbp", bufs=6) as vbp, \
             tc.tile_pool(name="grp", bufs=2) as grpp, \
             tc.tile_pool(name="sspp", bufs=16) as sspp, \
             tc.tile_pool(name="ps", bufs=2, space="PSUM") as ps:

            ident = single.tile([P, P], BF)
            make_identity(nc, ident)
            epst = single.tile([P, 1], F32)
            nc.vector.memset(epst, EPS)

            # ---- broadcast norm weights to all partitions ----
            qw_b = single.tile([P, D], F32)
            nc.sync.dma_start(out=qw_b, in_=qw2.ap().to_broadcast([P, D]))
            kw_b = single.tile([S, D], F32)
            nc.sync.dma_start(out=kw_b, in_=kw2.ap().to_broadcast([S, D]))

            # ---- rope cos/sin gathers ----
            pos_sb = single.tile([P, 1], I32, tag="pos0")
            pos_sb1 = single.tile([P, 1], I32, tag="pos1")
            pos_k = single.tile([S, 1], I32, tag="posk")
            nc.sync.dma_start(out=pos_sb, in_=posr[0:P, :])
            nc.sync.dma_start(out=pos_sb1, in_=posr[P:2 * P, :])
            nc.sync.dma_start(out=pos_k, in_=pos1[:, :])
            import concourse.bass as bass
            cq = [single.tile([P, H], F32, tag=f"cq{i}", name=f"cq{i}") for i in range(2)]
            sq = [single.tile([P, H], F32, tag=f"sq{i}", name=f"sq{i}") for i in range(2)]
            for i, pt_ in enumerate([pos_sb, pos_sb1]):
                nc.gpsimd.indirect_dma_start(
                    out=cq[i][:, :], out_offset=None, in_=cosc[:, :],
                    in_offset=bass.IndirectOffsetOnAxis(ap=pt_[:, 0:1], axis=0))
                nc.gpsimd.indirect_dma_start(
                    out=sq[i][:, :], out_offset=None, in_=sinc[:, :],
                    in_offset=bass.IndirectOffsetOnAxis(ap=pt_[:, 0:1], axis=0))
            ck = single.tile([S, H], F32, tag="ck")
            sk = single.tile([S, H], F32, tag="sk")
            nc.gpsimd.indirect_dma_start(
                out=ck[:, :], out_offset=None, in_=cosc[:, :],
                in_offset=bass.IndirectOffsetOnAxis(ap=pos_k[:, 0:1], axis=0))
            nc.gpsimd.indirect_dma_start(
                out=sk[:, :], out_offset=None, in_=sinc[:, :],
                in_offset=bass.IndirectOffsetOnAxis(ap=pos_k[:, 0:1], axis=0))

            def rmsnorm_rope(x, w_b, cos_t, sin_t, rows, tagp):
                """x: [rows, D] f32 tile -> returns bf16 [rows, D] roped tile."""
                ssq = single.tile([rows, 1], F32, tag=f"{tagp}ssq")
                sqs = single.tile([rows, D], F32, tag=f"{tagp}sqs")
                nc.scalar.activation(out=sqs, in_=x[:rows, :], func=AF.Square,
                                     accum_out=ssq[:, 0:1])
                rstd = single.tile([rows, 1], F32, tag=f"{tagp}rstd")
                nc.scalar.activation(out=rstd, in_=ssq, func=AF.Sqrt,
                                     scale=1.0 / D, bias=epst[:rows, 0:1])
                nc.vector.reciprocal(out=rstd, in_=rstd)
                xn = single.tile([rows, D], F32, tag=f"{tagp}xn")
                nc.vector.tensor_scalar(out=xn, in0=x[:rows, :], scalar1=rstd[:, 0:1],
                                        scalar2=None, op0=mybir.AluOpType.mult)
                nc.vector.tensor_tensor(out=xn, in0=xn, in1=w_b[:rows, :],
                                        op=mybir.AluOpType.mult)
                xr = single.tile([rows, D], F32, tag=f"{tagp}xr")
                tmp = single.tile([rows, H], F32, tag=f"{tagp}tmp")
                # x1*cos - x2*sin ; x2*cos + x1*sin
                nc.vector.tensor_tensor(out=xr[:, 0:H], in0=xn[:, 0:H], in1=cos_t[:rows, :], op=mybir.AluOpType.mult)
                nc.vector.tensor_tensor(out=tmp, in0=xn[:, H:D], in1=sin_t[:rows, :], op=mybir.AluOpType.mult)
                nc.vector.tensor_tensor(out=xr[:, 0:H], in0=xr[:, 0:H], in1=tmp, op=mybir.AluOpType.subtract)
                nc.vector.tensor_tensor(out=xr[:, H:D], in0=xn[:, H:D], in1=cos_t[:rows, :], op=mybir.AluOpType.mult)
                nc.vector.tensor_tensor(out=tmp, in0=xn[:, 0:H], in1=sin_t[:rows, :], op=mybir.AluOpType.mult)
                nc.vector.tensor_tensor(out=xr[:, H:D], in0=xr[:, H:D], in1=tmp, op=mybir.AluOpType.add)
                xbf = single.tile([rows, D], BF, tag=f"{tagp}xbf")
                nc.vector.tensor_copy(out=xbf, in_=xr)
                return xbf

            # ---- q prep: two [128, D] tiles of (s,g) rows -> qT per group ----
            qT = []
            for gi in range(2):
                qraw = single.tile([P, D], F32, tag=f"qraw{gi}")
                nc.sync.dma_start(out=qraw, in_=q_h[gi * P:(gi + 1) * P, :])
                qbf = rmsnorm_rope(qraw, qw_b, cq[gi], sq[gi], P, f"q{gi}")
                qtp = ps.tile([P, P], BF, tag="tp")
                nc.tensor.transpose(out=qtp, in_=qbf, identity=ident)
                qt = single.tile([P, P], BF, tag=f"qT{gi}")
                nc.vector.tensor_copy(out=qt, in_=qtp)
                qT.append(qt)

            # ---- k/v prep ----
            kraw = single.tile([S, D], F32, tag="kraw")
            nc.sync.dma_start(out=kraw, in_=k_h[:, :])
            k_bf = rmsnorm_rope(kraw, kw_b, ck, sk, S, "k")
            # k_bfT [d, s]: lets the per-seq new-token patch be a
            # partition-local DVE column copy into KT (no DMA, no stall)
            kbt_p = ps.tile([P, S], BF, tag="tp", name="kbt_p")
            nc.tensor.transpose(out=kbt_p, in_=k_bf[0:S, :], identity=ident[0:S, 0:S])
            k_bfT = single.tile([P, S], BF, tag="kbfT")
            nc.vector.tensor_copy(out=k_bfT, in_=kbt_p)
            vraw = single.tile([S, D], F32, tag="vraw")
            nc.sync.dma_start(out=vraw, in_=v_h[:, :])
            v_bf = single.tile([S, D], BF, tag="vbfp")
            nc.scalar.copy(out=v_bf, in_=vraw)

            # ---- main: 2 groups x 8 pairs x 4 seqs ----
            for grp in range(2):
                ssp_l = []
                e_g = grpp.tile([P, MAXLEN], BF, tag="e_g", name=f"e_g{grp}")
                for p in range(8):
                    spair = ps.tile([P, MAXLEN], F32, tag="spair")
                    esp = sb.tile([P, MAXLEN], BF, tag="esp")
                    ssp = sspp.tile([P, 1], F32, tag="ssp")
                    ssp_l.append(ssp)
                    seqs = [32 * grp + 4 * p + c for c in range(4)]
                    lmax = int(max(Ls[s] for s in seqs))
                    for c in range(4):
                        i = 4 * p + c          # seq index within group
                        s = seqs[c]
                        nts = int(nt[s])
                        width = nts * P
                        L = int(Ls[s])
                        # K tile load (f32 -> bf16 cast on SWDGE); the stale
                        # new-token row is fixed up in KT, not here
                        kbf = sb.tile([P, MAXLEN], BF, tag="kbf")
                        nc.gpsimd.dma_start(
                            out=kbf[:, 0:width].rearrange("t (j d) -> t j d", d=D),
                            in_=kc[int(row0[s]):int(row0[s]) + width, :].rearrange("(j t) d -> t j d", t=P),
                        )
                        # transpose K -> KT [d, t]
                        kts = ktsp.tile([P, MAXLEN], BF, tag="kts")
                        for jj in range(0, nts, 4):
                            jw = min(4, nts - jj)
                            ktp = ps.tile([P, 4 * P], BF, tag="tp")
                            for j2 in range(jw):
                                nc.tensor.transpose(
                                    out=ktp[:, j2 * P:(j2 + 1) * P],
                                    in_=kbf[:, (jj + j2) * P:(jj + j2 + 1) * P],
                                    identity=ident)
                            if (jj // 4) % 2 == 0:
                                nc.vector.tensor_copy(out=kts[:, jj * P:(jj + jw) * P],
                                                      in_=ktp[:, 0:jw * P])
                            else:
                                nc.scalar.copy(out=kts[:, jj * P:(jj + jw) * P],
                                               in_=ktp[:, 0:jw * P])
                        # patch the roped new-token key into KT column L-1
                        nc.vector.tensor_copy(out=kts[:, L - 1:L], in_=k_bfT[:, s:s + 1])
                        # scores
                        for ch in range(0, width, 512):
                            nw = min(512, width - ch)
                            nc.tensor.matmul(
                                out=spair[32 * c:32 * c + G, ch:ch + nw],
                                lhsT=qT[grp][:, G * i:G * i + G],
                                rhs=kts[:, ch:ch + nw],
                                start=True, stop=True,
                                tile_position=(0, 32 * c))
                        # mask junk scores in [L, lmax) so the pair-wide exp
                        # maps them to 0 (also kills PSUM garbage/NaN there)
                        if L < lmax:
                            nc.vector.memset(spair[32 * c:32 * c + G, L:lmax], -1e30)
                    # pair-wide exp: ACT is partition-parallel, so one op
                    # covers all 4 seqs; junk partitions are never read
                    if lmax < MAXLEN:
                        nc.vector.memset(esp[:, lmax:MAXLEN], 0.0)
                    nc.scalar.activation(
                        out=esp[:, 0:lmax],
                        in_=spair[:, 0:lmax],
                        func=AF.Exp, scale=float(SCALE),
                        accum_out=ssp[:, 0:1])
                    nc.vector.reciprocal(out=ssp_l[p], in_=ssp_l[p])
                    # densify this pair's E immediately so esp slots recycle
                    for b in range(4):
                        dma_eng = nc.sync if b % 2 == 0 else nc.scalar
                        dma_eng.dma_start(
                            out=e_g[4 * (4 * p + b):4 * (4 * p + b) + 4, :],
                            in_=esp[32 * b:32 * b + 4, :])
                # PT transposes: [t, (j, sg)], split by 64-row halves so the
                # first half's PV can start while the second half's scores run
                jmax = int(max(nt[32 * grp:32 * grp + 32]))
                pt_g = grpp.tile([P, MAXLEN], BF, tag="pt_g")
                for half in range(2):
                    hb = 64 * half
                    for jj in range(0, jmax, 4):
                        jw = min(4, jmax - jj)
                        ptp = ps.tile([P, 4 * P], BF, tag="tp", name=f"ptp{grp}{half}{jj}")
                        for j2 in range(jw):
                            nc.tensor.transpose(
                                out=ptp[:, j2 * 64:(j2 + 1) * 64],
                                in_=e_g[hb:hb + 64, (jj + j2) * P:(jj + j2 + 1) * P],
                                identity=ident[hb:hb + 64, hb:hb + 64])
                        nc.vector.tensor_copy(
                            out=pt_g[:, jj * P:(jj + jw) * P].rearrange(
                                "p (j x) -> p j x", x=P)[:, :, hb:hb + 64],
                            in_=ptp[:, 0:jw * 64].rearrange("p (j x) -> p j x", x=64))
                # PV + extraction + out
                for hg in range(2):                    # half-group: pairs 4hg..4hg+3
                    pvb = ps.tile([P, 512], F32, tag="pvb")
                    og = grpp.tile([P, 512], F32, tag="og")
                    for pc in range(4):
                        p = 4 * hg + pc
                        for c in range(4):
                            i = 4 * p + c
                            s = 32 * grp + i
                            nts = int(nt[s])
                            # V loads: full tiles + last-tile prefix + new-token
                            # row, all disjoint -> no ordering stalls
                            vbf = vbp.tile([P, MAXLEN], BF, tag="vbf")
                            full = (nts - 1) * P
                            ppv = int(pp[s])
                            r0 = int(row0[s])
                            if full > 0:
                                nc.gpsimd.dma_start(
                                    out=vbf[:, 0:full].rearrange("t (j d) -> t j d", d=D),
                                    in_=vc[r0:r0 + full, :].rearrange("(j t) d -> t j d", t=P),
                                )
                            # zero the last-tile region first: rows past the
                            # new token stay 0 (their probs are 0, but NaN
                            # garbage would poison the PSUM accumulation)
                            nc.vector.memset(vbf[:, full:full + D], 0.0)
                            if ppv > 0:
                                nc.gpsimd.dma_start(
                                    out=vbf[0:ppv, full:full + D],
                                    in_=vc[r0 + full:r0 + full + ppv, :],
                                )
                            nc.scalar.dma_start(
                                out=vbf[ppv:ppv + 1, full:full + D],
                                in_=v_bf[s:s + 1, :],
                            )
                            for j in range(nts):
                                nc.tensor.matmul(
                                    out=pvb[32 * c:32 * c + G, 128 * pc:128 * pc + D],
                                    lhsT=pt_g[:, j * P + G * i:j * P + G * i + G],
                                    rhs=vbf[:, j * P:(j + 1) * P],
                                    start=(j == 0), stop=(j == nts - 1),
                                    tile_position=(0, 32 * c))
                            # extraction with late softmax normalization
                            nc.vector.tensor_scalar(
                                out=og[32 * c:32 * c + 4, 128 * pc:128 * pc + D],
                                in0=pvb[32 * c:32 * c + 4, 128 * pc:128 * pc + D],
                                scalar1=ssp_l[p][32 * c:32 * c + 4, 0:1],
                                scalar2=None, op0=mybir.AluOpType.mult)
                    # out rows: 4*s+g = 128*grp + 64*hg + 16*pc + 4*c + g
                    r0 = 128 * grp + 64 * hg
                    o3 = outp.rearrange("(x y) d -> y x d", y=16)
                    for c in range(4):
                        nc.sync.dma_start(
                            out=o3[4 * c:4 * c + 4, r0 // 16:r0 // 16 + 4, :],
                            in_=og[32 * c:32 * c + 4, :].rearrange("g (pc d) -> g pc d", d=D))
    nc.compile()
    return nc


def kernel(q, k, v, k_cache, v_cache, qw, kw, cos_cache, sin_cache,
           position, slot_mapping, block_tables, context_lens):
    from concourse.bass_utils import run_bass_kernel_spmd

    q = np.asarray(q); k = np.asarray(k); v = np.asarray(v)
    k_cache = np.asarray(k_cache); v_cache = np.asarray(v_cache)
    qw = np.asarray(qw); kw = np.asarray(kw)
    cos_cache = np.asarray(cos_cache); sin_cache = np.asarray(sin_cache)
    position = np.asarray(position); slot_mapping = np.asarray(slot_mapping)
    block_tables = np.asarray(block_tables); context_lens = np.asarray(context_lens)

    L = context_lens.astype(np.int64)
    nt = (L + P - 1) // P
    pp = (L - 1) % P
    bt = block_tables.astype(np.int64)
    assert np.all(np.diff(bt, axis=1) == 1), "kernel assumes contiguous block tables"
    row0 = bt[:, 0] * BS
    assert np.all(slot_mapping.astype(np.int64) == row0 + L - 1), \
        "kernel assumes slot_mapping points at the last context position"

    key = (nt.tobytes(), pp.tobytes(), row0.tobytes(), L.tobytes())
    if key not in _cache:
        _cache[key] = _build(nt, pp, row0, L)
    nc = _cache[key]

    posr = np.repeat(position.astype(np.int32), G)[:, None]
    pos1 = position.astype(np.int32)[:, None]
    qr = np.ascontiguousarray(q.reshape(S, NH, D))          # [S, NH, D]
    kr = k.reshape(S, KVH, D)
    vr = v.reshape(S, KVH, D)
    in_maps = []
    for h in range(KVH):
        in_maps.append(dict(
            q_h=np.ascontiguousarray(qr[:, G * h:G * (h + 1), :]).reshape(S * G, D),
            k_h=np.ascontiguousarray(kr[:, h, :]),
            v_h=np.ascontiguousarray(vr[:, h, :]),
            kc=np.ascontiguousarray(k_cache[:, :, h, :]).reshape(S * MAXLEN, D),
            vc=np.ascontiguousarray(v_cache[:, :, h, :]).reshape(S * MAXLEN, D),
            qw2=np.ascontiguousarray(qw[None, :]),
            kw2=np.ascontiguousarray(kw[None, :]),
            cosc=np.ascontiguousarray(cos_cache),
            sinc=np.ascontiguousarray(sin_cache),
            posr=posr, pos1=pos1,
        ))
    global _last_in_maps
    _last_in_maps = in_maps
    res = run_bass_kernel_spmd(nc, in_maps, list(range(KVH)))
    # out per core: [S*G, D] -> full [S, NH*D]
    full = np.empty((S, NH, D), np.float32)
    for h in range(KVH):
        full[:, G * h:G * (h + 1), :] = res.results[h]["out"].reshape(S, G, D)
    return full.reshape(S, NH * D)

